# revision 55
# baseline (speedup 1.0000x reference)
"""Trainium2 Bass kernel for Bahdanau-style attention.

Reference computation (per batch column n):
    e  = tanh(hd @ W1.T + b1 + out_e @ W2.T + b2)      # [S, N, J]
    a  = e @ W3.T + b3                                 # [S, N, 1]
    alpha = softmax(a, axis=0)                         # [S, N, 1]
    c  = einsum('sne,snh->enh', alpha, out_e)          # [1, N, H]
    returns (c, alpha)

Sharding: data-parallel over batch dim N (64) across 8 cores (8 cols each).

Device-side layout (per core): everything runs TRANSPOSED.  The host packs
out_e as xT2[p, (n, sc, ht, so)] so each (n, sc) unit is ONE 2MB DMA with
16KB-per-partition lines (a single dma_start fans out over all 16 DMA
engines), and the big matmul z.T = W2 @ x.T contracts h on the partition
dim with W2.T blocks stationary (packed jt-major, loaded in four DMAs so
the first j-tiles' matmuls start early).  All matmuls use float32r (full
fp32 storage, reduced-precision multiply, 1 cycle/row at free>=256 vs 4
for plain fp32; measured end-to-end error ~2e-4).  Throwaway matmuls
warm the PE HAM clock-gate during the startup DMA.
The q-term (hd @ W1.T + b1 + b2, a [N, J]-sized bias independent of the
big tensor, 0.1% of total FLOPs) is precomputed host-side and folded into
the tanh as a per-partition bias (partition dim = j in the transposed
layout; each 512-wide chunk has a single fixed n).  The a-projection
contracts j on partitions with W3 as a [128, 1] stationary.  softmax(a)
== softmax(a+b3) so b3 is dropped; the logits are O(1) so the
max-subtraction is skipped and the context einsum runs on unnormalized
exp weights per 512-chunk (overlapping the other chunk's matmuls),
normalized at the [128, 1] accumulator level at the end.  exp replication
across partitions uses a K=1 ones matmul; the einsum itself is a fused
DVE multiply+accumulate.
"""

import os
import sys

sys.path.insert(0, "/opt/trn_rl_repo")
os.environ.setdefault("MYCRO_LOCAL_CACHE", "1")

import ml_dtypes
import numpy as np

import bass_rust
import concourse.bass as bass
import concourse.mybir as mybir
import concourse.tile as tile
from concourse.bass_utils import run_bass_kernel_spmd

S, N, H, J = 1024, 64, 1024, 1024
NCORES = 8
NLOC = N // NCORES          # 8 batch columns per core
HT = H // 128               # 8 h-tiles
JT = J // 128               # 8 j-tiles
M = NLOC * S                # 8192 rows per core

f32 = mybir.dt.float32
f32r = mybir.dt.float32r
bf16 = mybir.dt.bfloat16

N_PROCS = bass_rust.N_PROCS


class ChunkedTileContext(tile.TileContext):
    """TileContext variant for a walrus build that rejects >1 sync wait per
    instruction: splits multi-wait instructions into single-wait NoOp
    carriers and chunks the kernel-tail drain into one drain per proc."""

    def _split_multiwaits(self, ordered):
        for bb in list(ordered.keys()):
            out = []
            for inst in ordered[bb]:
                si = inst.sync_info
                if si is not None and len(si.on_wait) > 1:
                    waits = list(si.on_wait)
                    for w in waits[:-1]:
                        nop = mybir.InstNoOp(
                            name=f"wsplit{self.nc.next_id()}",
                            engine=inst.engine,
                            sync_info=mybir.SyncInfo(on_wait=[w], on_update=[]),
                            bass_nofuse=True,
                        )
                        out.append(nop)
                    inst.sync_info = mybir.SyncInfo(
                        on_wait=[waits[-1]], on_update=list(si.on_update)
                    )
                out.append(inst)
            ordered[bb] = out

    def _lower_ordered_insts(self, ordered):
        self._split_multiwaits(ordered)
        return super()._lower_ordered_insts(ordered)

    def _drain_and_barrier(self, tick_clock, wait_clock):
        gc = tick_clock.global_clock
        vals = [gc.peek_next(p) - 1 for p in range(N_PROCS)]
        for p, v in enumerate(vals):
            if v <= 0:
                continue
            partial = [0] * N_PROCS
            partial[p] = v
            d = self.nc.sync.drain()
            wait_clock.add_sem_waits(
                d.ins,
                bass_rust.ScopedClock({None: bass_rust.VectorClock(partial)}),
            )
        self.nc.sync.drain()
        self.nc.all_engine_barrier()
        assert self.sems is not None
        popped = self.nc._tile_sem_poison_stack.pop()
        assert popped is self._sem_poison
        self.nc.clear_and_free_semaphores(list(self.sems.allocated().values()))
        self.nc.all_engine_barrier()


def _build():
    nc = bass.Bass()

    # xT2[p, ((n*2 + sc)*HT + ht)*512 + so] = out_e[sc*512+so, n, ht*128+p]
    xT_d = nc.declare_dram_parameter("xT2", [128, NLOC * 2 * HT * 512], f32r,
                                     isOutput=False)
    # w2p[p, (jt*HT + ht)*128 + jj] = W2[jt*128+jj, ht*128+p]
    w2p_d = nc.declare_dram_parameter("w2p", [128, JT * HT * 128], f32r,
                                      isOutput=False)
    qt_d = nc.declare_dram_parameter("qt", [128, JT * NLOC], f32, isOutput=False)
    w3p_d = nc.declare_dram_parameter("w3p", [128, JT], f32r, isOutput=False)
    c_out_d = nc.declare_dram_parameter("c_out", [128, HT * NLOC], f32, isOutput=True)
    alpha_d = nc.declare_dram_parameter("alpha_out", [NLOC, S], f32, isOutput=True)

    Tanh = mybir.ActivationFunctionType.Tanh
    Exp = mybir.ActivationFunctionType.Exp
    mult = mybir.AluOpType.mult
    add = mybir.AluOpType.add

    with (
        ChunkedTileContext(nc) as tc,
        tc.tile_pool(name="const", bufs=1) as constp,
        tc.tile_pool(name="xtp", bufs=4) as xtp,
        tc.tile_pool(name="ttp", bufs=10) as ttp,
        tc.tile_pool(name="scrp", bufs=2) as scrp,
        tc.tile_pool(name="smp", bufs=2) as smp,
        tc.tile_pool(name="pz", bufs=5, space="PSUM") as pzp,
        tc.tile_pool(name="pa", bufs=1, space="PSUM") as pap,
        tc.tile_pool(name="pb", bufs=2, space="PSUM") as pbp,
    ):
        # ---------------- constants (tiny, arrive first) ----------------
        ones_f = constp.tile([1, 128], f32)
        nc.vector.memset(ones_f[:], 1.0)
        ones = constp.tile([1, 128], f32r)
        nc.vector.tensor_copy(ones[:], ones_f[:])
        qt_sb = constp.tile([128, JT * NLOC], f32)
        w3p_sb = constp.tile([128, JT], f32r)
        ct_sb = constp.tile([128, HT * NLOC], f32)

        # single big weight tile, jt-major; two DMA halves so jt 0-3 matmuls
        # can start while jt 4-7 weights stream
        w2_sb = constp.tile([128, JT * HT * 128], f32r)

        def w2blk(jt, ht):
            o = (jt * HT + ht) * 128
            return w2_sb[:, o:o + 128]

        # throwaway matmuls fill the PE during the startup DMA so the HAM
        # clock-gate is at 8/8 (and the pipe warm) when real data lands
        warm_in = constp.tile([128, 512], f32r)
        nc.vector.memset(warm_in[:].bitcast(f32), 0.0)
        pwarm = pzp.tile([128, 512], f32, tag="pz", name="pwarm")
        NWARM = 30
        for i in range(NWARM):
            nc.tensor.matmul(
                pwarm[:], warm_in[:, 0:128], warm_in[:],
                start=(i == 0), stop=(i == NWARM - 1),
            )

        # ---------------- main per-n pipeline ----------------
        for n in range(NLOC):
            xts = []
            if n == 0:
                # startup issue order tuned for the near-FIFO DMA drain:
                # w2 quarter 0 -> xt(0,0) -> qt/w3p -> remaining quarters
                # -> xt(0,1), so the first j-tiles' matmuls gate on 3MB and
                # the tanh bias arrives just in time
                for sc in range(2):
                    x = xtp.tile([128, HT * 512], f32r, tag="xt", name="xt")
                    xts.append(x)
                qsz = (JT // 4) * HT * 128
                nc.sync.dma_start(w2_sb[:, 0:qsz], w2p_d[:][:, 0:qsz])
                nc.sync.dma_start(xts[0][:], xT_d[:][:, 0:HT * 512])
                nc.sync.dma_start(qt_sb[:], qt_d[:])
                nc.sync.dma_start(w3p_sb[:], w3p_d[:])
                for quarter in range(1, 4):
                    o = quarter * qsz
                    nc.sync.dma_start(w2_sb[:, o:o + qsz], w2p_d[:][:, o:o + qsz])
                nc.sync.dma_start(
                    xts[1][:], xT_d[:][:, HT * 512:2 * HT * 512]
                )
            else:
                for sc in range(2):
                    x = xtp.tile([128, HT * 512], f32r, tag="xt", name="xt")
                    nc.sync.dma_start(
                        x[:],
                        xT_d[:][:, (n * 2 + sc) * HT * 512:(n * 2 + sc + 1) * HT * 512],
                    )
                    xts.append(x)

            # last n ends with two 256-wide chunks so the final post-matmul
            # (exp -> replicate -> einsum) tail is half as long.  Chunks in
            # the same group interleave per (jt, ht) so consecutive matmuls
            # share the stationary operand (saves ~12ns per shared reload);
            # n=0 keeps chunks in separate groups so the first group only
            # gates on the first 2MB of x.
            if n == NLOC - 1:
                groups = [[(0, 0, 512)], [(1, 0, 256)], [(1, 256, 256)]]
            else:
                groups = [[(0, 0, 512)], [(1, 0, 512)]]

            ses = []        # per-chunk (exp-values, exp-sum, base, width)
            tmps = []       # per-chunk list of per-ht unnormalized accums
            ci = 0
            for group in groups:
                all_tts = [[] for _ in group]
                for jt in range(JT):
                    pzs = [
                        pzp.tile([128, 512], f32, tag="pz", name="pz")
                        for _ in group
                    ]
                    for ht in range(HT):
                        for gi, (sc, off, w) in enumerate(group):
                            nc.tensor.matmul(
                                pzs[gi][:, 0:w],
                                w2blk(jt, ht),
                                xts[sc][:, ht * 512 + off:ht * 512 + off + w],
                                start=(ht == 0),
                                stop=(ht == HT - 1),
                            )
                    for gi, (sc, off, w) in enumerate(group):
                        tt = ttp.tile([128, 512], f32r, tag="tt", name="tt")
                        nc.scalar.activation(
                            tt[:, 0:w], pzs[gi][:, 0:w], Tanh,
                            bias=qt_sb[:, jt * NLOC + n:jt * NLOC + n + 1],
                            scale=1.0,
                        )
                        all_tts[gi].append(tt)
                for gi, (sc, off, w) in enumerate(group):
                    xsc = xts[sc]
                    tts = all_tts[gi]
                    pa = pap.tile([1, 512], f32, tag="pa", name="pa")
                    for jt in range(JT):
                        nc.tensor.matmul(
                            pa[:, 0:w],
                            w3p_sb[:, jt:jt + 1],
                            tts[jt][:, 0:w],
                            start=(jt == 0),
                            stop=(jt == JT - 1),
                        )
                    # unnormalized softmax weights: es = exp(a)  (logits are
                    # O(1): max-subtraction unnecessary; b3 cancels)
                    es = smp.tile([1, 512], f32r, tag="es", name="es", bufs=3)
                    se = smp.tile([1, 1], f32, tag="se", name="se", bufs=3)
                    nc.scalar.activation(
                        es[:, 0:w], pa[:, 0:w], Exp, scale=1.0, accum_out=se[:]
                    )
                    ses.append((es, se, sc * 512 + off, w))
                    # replicate es across 128 partitions via K=1 ones matmul
                    pb = pbp.tile([128, 512], f32, tag="pb", name="pb")
                    nc.tensor.matmul(
                        pb[:, 0:w], ones[:], es[:, 0:w], start=True, stop=True
                    )
                    # unnormalized context accumulation for this chunk
                    sc_tmps = []
                    for ht in range(HT):
                        tmp = smp.tile(
                            [128, 1], f32, tag=f"tmp{ci}", name="tmp", bufs=10
                        )
                        scr = scrp.tile([128, 512], f32, tag="scr", name="scr")
                        nc.vector.scalar_tensor_tensor(
                            out=scr[:, 0:w],
                            in0=xsc[:, ht * 512 + off:ht * 512 + off + w].bitcast(f32),
                            scalar=1.0, in1=pb[:, 0:w], op0=mult, op1=mult,
                            accum_out=tmp[:],
                        )
                        sc_tmps.append(tmp)
                    tmps.append(sc_tmps)
                    ci += 1

            # normalize: rs = 1 / sum(se), replicated 8-wide so the
            # partition-broadcast matmul has a valid free dim
            tot8 = smp.tile([1, 8], f32, tag="tot8", name="tot8")
            nc.vector.tensor_scalar(
                tot8[:], ones_f[:, 0:8], ses[0][1][:], ses[1][1][:],
                op0=mult, op1=add,
            )
            for es, se, _, _ in ses[2:]:
                nc.vector.tensor_scalar_add(tot8[:], tot8[:], se[:])
            rs_f = smp.tile([1, 8], f32, tag="rs_f", name="rs_f")
            nc.vector.reciprocal(rs_f[:], tot8[:])
            rs = smp.tile([1, 8], f32r, tag="rs", name="rs")
            nc.vector.tensor_copy(rs[:], rs_f[:])
            prs = pbp.tile([128, 8], f32, tag="pb", name="prs")
            nc.tensor.matmul(prs[:], ones[:], rs[:], start=True, stop=True)
            # alpha output rows: al = es * rs
            al = smp.tile([1, S], f32, tag="al", name="al")
            for es, se, base, w in ses:
                nc.vector.tensor_scalar_mul(
                    al[:, base:base + w], es[:, 0:w].bitcast(f32), rs_f[:, 0:1]
                )
            nc.sync.dma_start(alpha_d[:][n:n + 1, :], al[:])
            # ct column: sum(tmps) * rs
            for ht in range(HT):
                s01 = smp.tile([128, 1], f32, tag="s01", name="s01")
                nc.vector.tensor_tensor(s01[:], tmps[0][ht][:], tmps[1][ht][:], add)
                for ci in range(2, len(tmps)):
                    nc.vector.tensor_tensor(s01[:], s01[:], tmps[ci][ht][:], add)
                nc.vector.tensor_scalar_mul(
                    ct_sb[:, ht * NLOC + n:ht * NLOC + n + 1], s01[:], prs[:, 0:1]
                )

        nc.sync.dma_start(c_out_d[:], ct_sb[:])

    return nc


_CACHE = {}


def _get_nc():
    if "nc" not in _CACHE:
        _CACHE["nc"] = _build()
    return _CACHE["nc"]


def kernel(out_e, hidden_d, W1, b1, W2, b2, W3, b3):
    out_e = np.ascontiguousarray(out_e, dtype=np.float32)
    hidden_d = np.ascontiguousarray(hidden_d, dtype=np.float32)
    W1 = np.asarray(W1, np.float32)
    W2 = np.asarray(W2, np.float32)
    W3 = np.asarray(W3, np.float32)
    b1 = np.asarray(b1, np.float32)
    b2 = np.asarray(b2, np.float32)

    # w2p[p, (jt*HT + ht)*128 + jj] = W2[jt*128+jj, ht*128+p]
    w2p = np.ascontiguousarray(
        W2.reshape(JT, 128, HT, 128).transpose(3, 0, 2, 1).reshape(128, JT * HT * 128)
    )
    w3p = np.ascontiguousarray(W3[0].reshape(JT, 128).T)               # [128, JT]
    # q = hd @ W1.T + b1 + b2 : [N, J] bias term (0.1% of total FLOPs)
    q_full = hidden_d[0] @ W1.T + (b1 + b2)[None, :]

    in_maps = []
    for i in range(NCORES):
        sl = slice(i * NLOC, (i + 1) * NLOC)
        ne = out_e[:, sl, :]                                           # [S, NLOC, H]
        # xT2[p, n, sc, ht, so] = ne[sc*512+so, n, ht*128+p]
        xT2 = np.ascontiguousarray(
            ne.reshape(2, 512, NLOC, HT, 128).transpose(4, 2, 0, 3, 1)
        ).reshape(128, NLOC * 2 * HT * 512)
        q = q_full[sl]                                                 # [NLOC, J]
        qt = np.ascontiguousarray(
            q.T.reshape(JT, 128, NLOC).transpose(1, 0, 2).reshape(128, JT * NLOC)
        )
        in_maps.append({"xT2": xT2, "w2p": w2p, "qt": qt, "w3p": w3p})

    nc = _get_nc()
    res = run_bass_kernel_spmd(nc, in_maps, core_ids=list(range(NCORES)))

    c = np.empty((1, N, H), np.float32)
    alpha = np.empty((S, N, 1), np.float32)
    for i in range(NCORES):
        ct = res.results[i]["c_out"]                                   # [128, HT*NLOC]
        c_i = ct.reshape(128, HT, NLOC).transpose(2, 1, 0).reshape(NLOC, H)
        c[0, i * NLOC:(i + 1) * NLOC, :] = c_i
        alpha[:, i * NLOC:(i + 1) * NLOC, 0] = res.results[i]["alpha_out"].T
    return c, alpha


# revision 56
# speedup vs baseline: 1.0004x; 1.0004x over previous
"""Trainium2 Bass kernel for Bahdanau-style attention.

Reference computation (per batch column n):
    e  = tanh(hd @ W1.T + b1 + out_e @ W2.T + b2)      # [S, N, J]
    a  = e @ W3.T + b3                                 # [S, N, 1]
    alpha = softmax(a, axis=0)                         # [S, N, 1]
    c  = einsum('sne,snh->enh', alpha, out_e)          # [1, N, H]
    returns (c, alpha)

Sharding: data-parallel over batch dim N (64) across 8 cores (8 cols each).

Device-side layout (per core): everything runs TRANSPOSED.  The host packs
out_e as xT2[p, (n, sc, ht, so)] so each (n, sc) unit is ONE 2MB DMA with
16KB-per-partition lines (a single dma_start fans out over all 16 DMA
engines), and the big matmul z.T = W2 @ x.T contracts h on the partition
dim with W2.T blocks stationary (packed jt-major, loaded in four DMAs so
the first j-tiles' matmuls start early).  All matmuls use float32r (full
fp32 storage, reduced-precision multiply, 1 cycle/row at free>=256 vs 4
for plain fp32; measured end-to-end error ~2e-4).  Throwaway matmuls
warm the PE HAM clock-gate during the startup DMA.
The q-term (hd @ W1.T + b1 + b2, a [N, J]-sized bias independent of the
big tensor, 0.1% of total FLOPs) is precomputed host-side and folded into
the tanh as a per-partition bias (partition dim = j in the transposed
layout; each 512-wide chunk has a single fixed n).  The a-projection
contracts j on partitions with W3 as a [128, 1] stationary.  softmax(a)
== softmax(a+b3) so b3 is dropped; the logits are O(1) so the
max-subtraction is skipped and the context einsum runs on unnormalized
exp weights per 512-chunk (overlapping the other chunk's matmuls),
normalized at the [128, 1] accumulator level at the end.  exp replication
across partitions uses a K=1 ones matmul; the einsum itself is a fused
DVE multiply+accumulate.
"""

import os
import sys

sys.path.insert(0, "/opt/trn_rl_repo")
os.environ.setdefault("MYCRO_LOCAL_CACHE", "1")

import ml_dtypes
import numpy as np

import bass_rust
import concourse.bass as bass
import concourse.mybir as mybir
import concourse.tile as tile
from concourse.bass_utils import run_bass_kernel_spmd

S, N, H, J = 1024, 64, 1024, 1024
NCORES = 8
NLOC = N // NCORES          # 8 batch columns per core
HT = H // 128               # 8 h-tiles
JT = J // 128               # 8 j-tiles
M = NLOC * S                # 8192 rows per core

f32 = mybir.dt.float32
f32r = mybir.dt.float32r
bf16 = mybir.dt.bfloat16

N_PROCS = bass_rust.N_PROCS


class ChunkedTileContext(tile.TileContext):
    """TileContext variant for a walrus build that rejects >1 sync wait per
    instruction: splits multi-wait instructions into single-wait NoOp
    carriers and chunks the kernel-tail drain into one drain per proc."""

    def _split_multiwaits(self, ordered):
        for bb in list(ordered.keys()):
            out = []
            for inst in ordered[bb]:
                si = inst.sync_info
                if si is not None and len(si.on_wait) > 1:
                    waits = list(si.on_wait)
                    for w in waits[:-1]:
                        nop = mybir.InstNoOp(
                            name=f"wsplit{self.nc.next_id()}",
                            engine=inst.engine,
                            sync_info=mybir.SyncInfo(on_wait=[w], on_update=[]),
                            bass_nofuse=True,
                        )
                        out.append(nop)
                    inst.sync_info = mybir.SyncInfo(
                        on_wait=[waits[-1]], on_update=list(si.on_update)
                    )
                out.append(inst)
            ordered[bb] = out

    def _lower_ordered_insts(self, ordered):
        self._split_multiwaits(ordered)
        return super()._lower_ordered_insts(ordered)

    def _drain_and_barrier(self, tick_clock, wait_clock):
        gc = tick_clock.global_clock
        vals = [gc.peek_next(p) - 1 for p in range(N_PROCS)]
        for p, v in enumerate(vals):
            if v <= 0:
                continue
            partial = [0] * N_PROCS
            partial[p] = v
            d = self.nc.sync.drain()
            wait_clock.add_sem_waits(
                d.ins,
                bass_rust.ScopedClock({None: bass_rust.VectorClock(partial)}),
            )
        self.nc.sync.drain()
        self.nc.all_engine_barrier()
        assert self.sems is not None
        popped = self.nc._tile_sem_poison_stack.pop()
        assert popped is self._sem_poison
        self.nc.clear_and_free_semaphores(list(self.sems.allocated().values()))
        self.nc.all_engine_barrier()


def _build():
    nc = bass.Bass()

    # xT2[p, ((n*2 + sc)*HT + ht)*512 + so] = out_e[sc*512+so, n, ht*128+p]
    xT_d = nc.declare_dram_parameter("xT2", [128, NLOC * 2 * HT * 512], f32r,
                                     isOutput=False)
    # w2p[p, (jt*HT + ht)*128 + jj] = W2[jt*128+jj, ht*128+p]
    w2p_d = nc.declare_dram_parameter("w2p", [128, JT * HT * 128], f32r,
                                      isOutput=False)
    qt_d = nc.declare_dram_parameter("qt", [128, JT * NLOC], f32, isOutput=False)
    w3p_d = nc.declare_dram_parameter("w3p", [128, JT], f32r, isOutput=False)
    c_out_d = nc.declare_dram_parameter("c_out", [128, HT * NLOC], f32, isOutput=True)
    alpha_d = nc.declare_dram_parameter("alpha_out", [NLOC, S], f32, isOutput=True)

    Tanh = mybir.ActivationFunctionType.Tanh
    Exp = mybir.ActivationFunctionType.Exp
    mult = mybir.AluOpType.mult
    add = mybir.AluOpType.add

    with (
        ChunkedTileContext(nc) as tc,
        tc.tile_pool(name="const", bufs=1) as constp,
        tc.tile_pool(name="xtp", bufs=4) as xtp,
        tc.tile_pool(name="ttp", bufs=10) as ttp,
        tc.tile_pool(name="scrp", bufs=2) as scrp,
        tc.tile_pool(name="smp", bufs=2) as smp,
        tc.tile_pool(name="pz", bufs=5, space="PSUM") as pzp,
        tc.tile_pool(name="pa", bufs=1, space="PSUM") as pap,
        tc.tile_pool(name="pb", bufs=2, space="PSUM") as pbp,
    ):
        # ---------------- constants (tiny, arrive first) ----------------
        ones_f = constp.tile([1, 128], f32)
        nc.vector.memset(ones_f[:], 1.0)
        ones = constp.tile([1, 128], f32r)
        nc.vector.tensor_copy(ones[:], ones_f[:])
        qt_sb = constp.tile([128, JT * NLOC], f32)
        nc.sync.dma_start(qt_sb[:], qt_d[:])
        w3p_sb = constp.tile([128, JT], f32r)
        nc.sync.dma_start(w3p_sb[:], w3p_d[:])
        ct_sb = constp.tile([128, HT * NLOC], f32)

        # single big weight tile, jt-major; two DMA halves so jt 0-3 matmuls
        # can start while jt 4-7 weights stream
        w2_sb = constp.tile([128, JT * HT * 128], f32r)

        def w2blk(jt, ht):
            o = (jt * HT + ht) * 128
            return w2_sb[:, o:o + 128]

        # throwaway matmuls fill the PE during the startup DMA so the HAM
        # clock-gate is at 8/8 (and the pipe warm) when real data lands
        warm_in = constp.tile([128, 512], f32r)
        nc.vector.memset(warm_in[:].bitcast(f32), 0.0)
        pwarm = pzp.tile([128, 512], f32, tag="pz", name="pwarm")
        NWARM = 40
        for i in range(NWARM):
            nc.tensor.matmul(
                pwarm[:], warm_in[:, 0:128], warm_in[:],
                start=(i == 0), stop=(i == NWARM - 1),
            )

        # ---------------- main per-n pipeline ----------------
        for n in range(NLOC):
            xts = []
            for sc in range(2):
                x = xtp.tile([128, HT * 512], f32r, tag="xt", name="xt")
                nc.sync.dma_start(
                    x[:],
                    xT_d[:][:, (n * 2 + sc) * HT * 512:(n * 2 + sc + 1) * HT * 512],
                )
                xts.append(x)
                if n == 0 and sc == 0:
                    for quarter in range(4):
                        o = quarter * (JT // 4) * HT * 128
                        sz = (JT // 4) * HT * 128
                        nc.sync.dma_start(
                            w2_sb[:, o:o + sz], w2p_d[:][:, o:o + sz]
                        )

            # last n ends with two 256-wide chunks so the final post-matmul
            # (exp -> replicate -> einsum) tail is half as long.  Chunks in
            # the same group interleave per (jt, ht) so consecutive matmuls
            # share the stationary operand (saves ~12ns per shared reload);
            # n=0 keeps chunks in separate groups so the first group only
            # gates on the first 2MB of x.
            if n == NLOC - 1:
                groups = [[(0, 0, 512)], [(1, 0, 256)], [(1, 256, 256)]]
            else:
                groups = [[(0, 0, 512)], [(1, 0, 512)]]

            ses = []        # per-chunk (exp-values, exp-sum, base, width)
            tmps = []       # per-chunk list of per-ht unnormalized accums
            ci = 0
            for group in groups:
                all_tts = [[] for _ in group]
                for jt in range(JT):
                    pzs = [
                        pzp.tile([128, 512], f32, tag="pz", name="pz")
                        for _ in group
                    ]
                    for ht in range(HT):
                        for gi, (sc, off, w) in enumerate(group):
                            nc.tensor.matmul(
                                pzs[gi][:, 0:w],
                                w2blk(jt, ht),
                                xts[sc][:, ht * 512 + off:ht * 512 + off + w],
                                start=(ht == 0),
                                stop=(ht == HT - 1),
                            )
                    for gi, (sc, off, w) in enumerate(group):
                        tt = ttp.tile([128, 512], f32r, tag="tt", name="tt")
                        nc.scalar.activation(
                            tt[:, 0:w], pzs[gi][:, 0:w], Tanh,
                            bias=qt_sb[:, jt * NLOC + n:jt * NLOC + n + 1],
                            scale=1.0,
                        )
                        all_tts[gi].append(tt)
                for gi, (sc, off, w) in enumerate(group):
                    xsc = xts[sc]
                    tts = all_tts[gi]
                    pa = pap.tile([1, 512], f32, tag="pa", name="pa")
                    for jt in range(JT):
                        nc.tensor.matmul(
                            pa[:, 0:w],
                            w3p_sb[:, jt:jt + 1],
                            tts[jt][:, 0:w],
                            start=(jt == 0),
                            stop=(jt == JT - 1),
                        )
                    # unnormalized softmax weights: es = exp(a)  (logits are
                    # O(1): max-subtraction unnecessary; b3 cancels)
                    es = smp.tile([1, 512], f32r, tag="es", name="es", bufs=3)
                    se = smp.tile([1, 1], f32, tag="se", name="se", bufs=3)
                    nc.scalar.activation(
                        es[:, 0:w], pa[:, 0:w], Exp, scale=1.0, accum_out=se[:]
                    )
                    ses.append((es, se, sc * 512 + off, w))
                    # replicate es across 128 partitions via K=1 ones matmul
                    pb = pbp.tile([128, 512], f32, tag="pb", name="pb")
                    nc.tensor.matmul(
                        pb[:, 0:w], ones[:], es[:, 0:w], start=True, stop=True
                    )
                    # unnormalized context accumulation for this chunk
                    sc_tmps = []
                    for ht in range(HT):
                        tmp = smp.tile(
                            [128, 1], f32, tag=f"tmp{ci}", name="tmp", bufs=10
                        )
                        scr = scrp.tile([128, 512], f32, tag="scr", name="scr")
                        nc.vector.scalar_tensor_tensor(
                            out=scr[:, 0:w],
                            in0=xsc[:, ht * 512 + off:ht * 512 + off + w].bitcast(f32),
                            scalar=1.0, in1=pb[:, 0:w], op0=mult, op1=mult,
                            accum_out=tmp[:],
                        )
                        sc_tmps.append(tmp)
                    tmps.append(sc_tmps)
                    ci += 1

            # normalize: rs = 1 / sum(se), replicated 8-wide so the
            # partition-broadcast matmul has a valid free dim
            tot8 = smp.tile([1, 8], f32, tag="tot8", name="tot8")
            nc.vector.tensor_scalar(
                tot8[:], ones_f[:, 0:8], ses[0][1][:], ses[1][1][:],
                op0=mult, op1=add,
            )
            for es, se, _, _ in ses[2:]:
                nc.vector.tensor_scalar_add(tot8[:], tot8[:], se[:])
            rs_f = smp.tile([1, 8], f32, tag="rs_f", name="rs_f")
            nc.vector.reciprocal(rs_f[:], tot8[:])
            rs = smp.tile([1, 8], f32r, tag="rs", name="rs")
            nc.vector.tensor_copy(rs[:], rs_f[:])
            prs = pbp.tile([128, 8], f32, tag="pb", name="prs")
            nc.tensor.matmul(prs[:], ones[:], rs[:], start=True, stop=True)
            # alpha output rows: al = es * rs
            al = smp.tile([1, S], f32, tag="al", name="al")
            for es, se, base, w in ses:
                nc.vector.tensor_scalar_mul(
                    al[:, base:base + w], es[:, 0:w].bitcast(f32), rs_f[:, 0:1]
                )
            nc.sync.dma_start(alpha_d[:][n:n + 1, :], al[:])
            # ct column: sum(tmps) * rs
            for ht in range(HT):
                s01 = smp.tile([128, 1], f32, tag="s01", name="s01")
                nc.vector.tensor_tensor(s01[:], tmps[0][ht][:], tmps[1][ht][:], add)
                for ci in range(2, len(tmps)):
                    nc.vector.tensor_tensor(s01[:], s01[:], tmps[ci][ht][:], add)
                nc.vector.tensor_scalar_mul(
                    ct_sb[:, ht * NLOC + n:ht * NLOC + n + 1], s01[:], prs[:, 0:1]
                )

        nc.sync.dma_start(c_out_d[:], ct_sb[:])

    return nc


_CACHE = {}


def _get_nc():
    if "nc" not in _CACHE:
        _CACHE["nc"] = _build()
    return _CACHE["nc"]


def kernel(out_e, hidden_d, W1, b1, W2, b2, W3, b3):
    out_e = np.ascontiguousarray(out_e, dtype=np.float32)
    hidden_d = np.ascontiguousarray(hidden_d, dtype=np.float32)
    W1 = np.asarray(W1, np.float32)
    W2 = np.asarray(W2, np.float32)
    W3 = np.asarray(W3, np.float32)
    b1 = np.asarray(b1, np.float32)
    b2 = np.asarray(b2, np.float32)

    # w2p[p, (jt*HT + ht)*128 + jj] = W2[jt*128+jj, ht*128+p]
    w2p = np.ascontiguousarray(
        W2.reshape(JT, 128, HT, 128).transpose(3, 0, 2, 1).reshape(128, JT * HT * 128)
    )
    w3p = np.ascontiguousarray(W3[0].reshape(JT, 128).T)               # [128, JT]
    # q = hd @ W1.T + b1 + b2 : [N, J] bias term (0.1% of total FLOPs)
    q_full = hidden_d[0] @ W1.T + (b1 + b2)[None, :]

    in_maps = []
    for i in range(NCORES):
        sl = slice(i * NLOC, (i + 1) * NLOC)
        ne = out_e[:, sl, :]                                           # [S, NLOC, H]
        # xT2[p, n, sc, ht, so] = ne[sc*512+so, n, ht*128+p]
        xT2 = np.ascontiguousarray(
            ne.reshape(2, 512, NLOC, HT, 128).transpose(4, 2, 0, 3, 1)
        ).reshape(128, NLOC * 2 * HT * 512)
        q = q_full[sl]                                                 # [NLOC, J]
        qt = np.ascontiguousarray(
            q.T.reshape(JT, 128, NLOC).transpose(1, 0, 2).reshape(128, JT * NLOC)
        )
        in_maps.append({"xT2": xT2, "w2p": w2p, "qt": qt, "w3p": w3p})

    nc = _get_nc()
    res = run_bass_kernel_spmd(nc, in_maps, core_ids=list(range(NCORES)))

    c = np.empty((1, N, H), np.float32)
    alpha = np.empty((S, N, 1), np.float32)
    for i in range(NCORES):
        ct = res.results[i]["c_out"]                                   # [128, HT*NLOC]
        c_i = ct.reshape(128, HT, NLOC).transpose(2, 1, 0).reshape(NLOC, H)
        c[0, i * NLOC:(i + 1) * NLOC, :] = c_i
        alpha[:, i * NLOC:(i + 1) * NLOC, 0] = res.results[i]["alpha_out"].T
    return c, alpha


# revision 58
# speedup vs baseline: 1.0019x; 1.0016x over previous
"""Trainium2 Bass kernel for Bahdanau-style attention.

Reference computation (per batch column n):
    e  = tanh(hd @ W1.T + b1 + out_e @ W2.T + b2)      # [S, N, J]
    a  = e @ W3.T + b3                                 # [S, N, 1]
    alpha = softmax(a, axis=0)                         # [S, N, 1]
    c  = einsum('sne,snh->enh', alpha, out_e)          # [1, N, H]
    returns (c, alpha)

Sharding: data-parallel over batch dim N (64) across 8 cores (8 cols each).

Device-side layout (per core): everything runs TRANSPOSED.  The host packs
out_e as xT2[p, (n, sc, ht, so)] so each (n, sc) unit is ONE 2MB DMA with
16KB-per-partition lines (a single dma_start fans out over all 16 DMA
engines), and the big matmul z.T = W2 @ x.T contracts h on the partition
dim with W2.T blocks stationary (packed jt-major, loaded in four DMAs so
the first j-tiles' matmuls start early).  All matmuls use float32r (full
fp32 storage, reduced-precision multiply, 1 cycle/row at free>=256 vs 4
for plain fp32; measured end-to-end error ~2e-4).  Throwaway matmuls
warm the PE HAM clock-gate during the startup DMA.
The q-term (hd @ W1.T + b1 + b2, a [N, J]-sized bias independent of the
big tensor, 0.1% of total FLOPs) is precomputed host-side and folded into
the tanh as a per-partition bias (partition dim = j in the transposed
layout; each 512-wide chunk has a single fixed n).  The a-projection
contracts j on partitions with W3 as a [128, 1] stationary.  softmax(a)
== softmax(a+b3) so b3 is dropped; the logits are O(1) so the
max-subtraction is skipped and the context einsum runs on unnormalized
exp weights per 512-chunk (overlapping the other chunk's matmuls),
normalized at the [128, 1] accumulator level at the end.  exp replication
across partitions uses a K=1 ones matmul; the einsum itself is a fused
DVE multiply+accumulate.
"""

import os
import sys

sys.path.insert(0, "/opt/trn_rl_repo")
os.environ.setdefault("MYCRO_LOCAL_CACHE", "1")

import ml_dtypes
import numpy as np

import bass_rust
import concourse.bass as bass
import concourse.mybir as mybir
import concourse.tile as tile
from concourse.bass_utils import run_bass_kernel_spmd

S, N, H, J = 1024, 64, 1024, 1024
NCORES = 8
NLOC = N // NCORES          # 8 batch columns per core
HT = H // 128               # 8 h-tiles
JT = J // 128               # 8 j-tiles
M = NLOC * S                # 8192 rows per core

f32 = mybir.dt.float32
f32r = mybir.dt.float32r
bf16 = mybir.dt.bfloat16

N_PROCS = bass_rust.N_PROCS


class ChunkedTileContext(tile.TileContext):
    """TileContext variant for a walrus build that rejects >1 sync wait per
    instruction: splits multi-wait instructions into single-wait NoOp
    carriers and chunks the kernel-tail drain into one drain per proc."""

    def _split_multiwaits(self, ordered):
        for bb in list(ordered.keys()):
            out = []
            for inst in ordered[bb]:
                si = inst.sync_info
                if si is not None and len(si.on_wait) > 1:
                    waits = list(si.on_wait)
                    for w in waits[:-1]:
                        nop = mybir.InstNoOp(
                            name=f"wsplit{self.nc.next_id()}",
                            engine=inst.engine,
                            sync_info=mybir.SyncInfo(on_wait=[w], on_update=[]),
                            bass_nofuse=True,
                        )
                        out.append(nop)
                    inst.sync_info = mybir.SyncInfo(
                        on_wait=[waits[-1]], on_update=list(si.on_update)
                    )
                out.append(inst)
            ordered[bb] = out

    def _lower_ordered_insts(self, ordered):
        self._split_multiwaits(ordered)
        return super()._lower_ordered_insts(ordered)

    def _drain_and_barrier(self, tick_clock, wait_clock):
        gc = tick_clock.global_clock
        vals = [gc.peek_next(p) - 1 for p in range(N_PROCS)]
        for p, v in enumerate(vals):
            if v <= 0:
                continue
            partial = [0] * N_PROCS
            partial[p] = v
            d = self.nc.sync.drain()
            wait_clock.add_sem_waits(
                d.ins,
                bass_rust.ScopedClock({None: bass_rust.VectorClock(partial)}),
            )
        self.nc.sync.drain()
        self.nc.all_engine_barrier()
        assert self.sems is not None
        popped = self.nc._tile_sem_poison_stack.pop()
        assert popped is self._sem_poison
        self.nc.clear_and_free_semaphores(list(self.sems.allocated().values()))
        self.nc.all_engine_barrier()


def _build():
    nc = bass.Bass()

    # xT2[p, ((n*2 + sc)*HT + ht)*512 + so] = out_e[sc*512+so, n, ht*128+p]
    xT_d = nc.declare_dram_parameter("xT2", [128, NLOC * 2 * HT * 512], f32r,
                                     isOutput=False)
    # w2p[p, (jt*HT + ht)*128 + jj] = W2[jt*128+jj, ht*128+p]
    w2p_d = nc.declare_dram_parameter("w2p", [128, JT * HT * 128], f32r,
                                      isOutput=False)
    qt_d = nc.declare_dram_parameter("qt", [128, JT * NLOC], f32, isOutput=False)
    w3p_d = nc.declare_dram_parameter("w3p", [128, JT], f32r, isOutput=False)
    c_out_d = nc.declare_dram_parameter("c_out", [128, HT * NLOC], f32, isOutput=True)
    alpha_d = nc.declare_dram_parameter("alpha_out", [NLOC, S], f32, isOutput=True)

    Tanh = mybir.ActivationFunctionType.Tanh
    Exp = mybir.ActivationFunctionType.Exp
    mult = mybir.AluOpType.mult
    add = mybir.AluOpType.add

    with (
        ChunkedTileContext(nc) as tc,
        tc.tile_pool(name="const", bufs=1) as constp,
        tc.tile_pool(name="xtp", bufs=4) as xtp,
        tc.tile_pool(name="ttp", bufs=10) as ttp,
        tc.tile_pool(name="scrp", bufs=2) as scrp,
        tc.tile_pool(name="smp", bufs=2) as smp,
        tc.tile_pool(name="pz", bufs=5, space="PSUM") as pzp,
        tc.tile_pool(name="pa", bufs=1, space="PSUM") as pap,
        tc.tile_pool(name="pb", bufs=2, space="PSUM") as pbp,
    ):
        # ---------------- constants (tiny, arrive first) ----------------
        ones_f = constp.tile([1, 128], f32)
        nc.vector.memset(ones_f[:], 1.0)
        ones = constp.tile([1, 128], f32r)
        nc.vector.tensor_copy(ones[:], ones_f[:])
        qt_sb = constp.tile([128, JT * NLOC], f32)
        nc.sync.dma_start(qt_sb[:], qt_d[:])
        w3p_sb = constp.tile([128, JT], f32r)
        nc.sync.dma_start(w3p_sb[:], w3p_d[:])
        ct_sb = constp.tile([128, HT * NLOC], f32)

        # single big weight tile, jt-major; two DMA halves so jt 0-3 matmuls
        # can start while jt 4-7 weights stream
        w2_sb = constp.tile([128, JT * HT * 128], f32r)

        def w2blk(jt, ht):
            o = (jt * HT + ht) * 128
            return w2_sb[:, o:o + 128]

        # throwaway matmuls fill the PE during the startup DMA so the HAM
        # clock-gate is at 8/8 (and the pipe warm) when real data lands
        warm_in = constp.tile([128, 512], f32r)
        nc.vector.memset(warm_in[:].bitcast(f32), 0.0)
        pwarm = pzp.tile([128, 512], f32, tag="pz", name="pwarm")
        NWARM = 40
        for i in range(NWARM):
            nc.tensor.matmul(
                pwarm[:], warm_in[:, 0:128], warm_in[:],
                start=(i == 0), stop=(i == NWARM - 1),
            )

        # ---------------- main per-n pipeline ----------------
        for n in range(NLOC):
            xts = []
            for sc in range(2):
                x = xtp.tile([128, HT * 512], f32r, tag="xt", name="xt")
                nc.sync.dma_start(
                    x[:],
                    xT_d[:][:, (n * 2 + sc) * HT * 512:(n * 2 + sc + 1) * HT * 512],
                )
                xts.append(x)
                if n == 0 and sc == 0:
                    for quarter in range(4):
                        o = quarter * (JT // 4) * HT * 128
                        sz = (JT // 4) * HT * 128
                        nc.sync.dma_start(
                            w2_sb[:, o:o + sz], w2p_d[:][:, o:o + sz]
                        )

            # last n ends with two 256-wide chunks so the final post-matmul
            # (exp -> replicate -> einsum) tail is half as long.  Chunks in
            # the same group interleave per (jt, ht) so consecutive matmuls
            # share the stationary operand (saves ~12ns per shared reload);
            # n=0 keeps chunks in separate groups so the first group only
            # gates on the first 2MB of x.
            if n == NLOC - 1:
                groups = [[(0, 0, 512)], [(1, 0, 256)], [(1, 256, 256)]]
            else:
                groups = [[(0, 0, 512)], [(1, 0, 512)]]

            ses = []        # per-chunk (exp-values, exp-sum, base, width)
            tmps = []       # per-chunk list of per-ht unnormalized accums
            ci = 0
            for group in groups:
                all_tts = [[] for _ in group]
                for jt in range(JT):
                    pzs = [
                        pzp.tile([128, 512], f32, tag="pz", name="pz")
                        for _ in group
                    ]
                    for ht in range(HT):
                        for gi, (sc, off, w) in enumerate(group):
                            nc.tensor.matmul(
                                pzs[gi][:, 0:w],
                                w2blk(jt, ht),
                                xts[sc][:, ht * 512 + off:ht * 512 + off + w],
                                start=(ht == 0),
                                stop=(ht == HT - 1),
                            )
                    for gi, (sc, off, w) in enumerate(group):
                        tt = ttp.tile([128, 512], f32r, tag="tt", name="tt")
                        nc.scalar.activation(
                            tt[:, 0:w], pzs[gi][:, 0:w], Tanh,
                            bias=qt_sb[:, jt * NLOC + n:jt * NLOC + n + 1],
                            scale=1.0,
                        )
                        all_tts[gi].append(tt)
                for gi, (sc, off, w) in enumerate(group):
                    xsc = xts[sc]
                    tts = all_tts[gi]
                    pa = pap.tile([1, 512], f32, tag="pa", name="pa")
                    for jt in range(JT):
                        nc.tensor.matmul(
                            pa[:, 0:w],
                            w3p_sb[:, jt:jt + 1],
                            tts[jt][:, 0:w],
                            start=(jt == 0),
                            stop=(jt == JT - 1),
                        )
                    # unnormalized softmax weights: es = exp(a)  (logits are
                    # O(1): max-subtraction unnecessary; b3 cancels)
                    es = smp.tile([1, 512], f32r, tag="es", name="es", bufs=3)
                    se = smp.tile([1, 1], f32, tag="se", name="se", bufs=3)
                    nc.scalar.activation(
                        es[:, 0:w], pa[:, 0:w], Exp, scale=1.0, accum_out=se[:]
                    )
                    ses.append((es, se, sc * 512 + off, w))
                    # replicate es across 128 partitions via K=1 ones matmul
                    pb = pbp.tile([128, 512], f32, tag="pb", name="pb")
                    nc.tensor.matmul(
                        pb[:, 0:w], ones[:], es[:, 0:w], start=True, stop=True
                    )
                    # unnormalized context accumulation for this chunk
                    sc_tmps = []
                    for ht in range(HT):
                        tmp = smp.tile(
                            [128, 1], f32, tag=f"tmp{ci}", name="tmp", bufs=10
                        )
                        scr = scrp.tile([128, 512], f32, tag="scr", name="scr")
                        nc.vector.scalar_tensor_tensor(
                            out=scr[:, 0:w],
                            in0=xsc[:, ht * 512 + off:ht * 512 + off + w].bitcast(f32),
                            scalar=1.0, in1=pb[:, 0:w], op0=mult, op1=mult,
                            accum_out=tmp[:],
                        )
                        sc_tmps.append(tmp)
                    tmps.append(sc_tmps)
                    ci += 1

            # normalize: rs = 1 / sum(se), replicated 8-wide so the
            # partition-broadcast matmul has a valid free dim
            tot8 = smp.tile([1, 8], f32, tag="tot8", name="tot8")
            nc.vector.tensor_scalar(
                tot8[:], ones_f[:, 0:8], ses[0][1][:], ses[1][1][:],
                op0=mult, op1=add,
            )
            for es, se, _, _ in ses[2:]:
                nc.vector.tensor_scalar_add(tot8[:], tot8[:], se[:])
            rs_f = smp.tile([1, 8], f32, tag="rs_f", name="rs_f")
            nc.vector.reciprocal(rs_f[:], tot8[:])
            rs = smp.tile([1, 8], f32r, tag="rs", name="rs")
            nc.vector.tensor_copy(rs[:], rs_f[:])
            prs = pbp.tile([128, 8], f32, tag="pb", name="prs")
            nc.tensor.matmul(prs[:], ones[:], rs[:], start=True, stop=True)
            # alpha output rows: al = es * rs
            al = smp.tile([1, S], f32, tag="al", name="al")
            for es, se, base, w in ses:
                nc.vector.tensor_scalar_mul(
                    al[:, base:base + w], es[:, 0:w].bitcast(f32), rs_f[:, 0:1]
                )
            nc.sync.dma_start(alpha_d[:][n:n + 1, :], al[:])
            # ct column: sum(tmps) * rs
            for ht in range(HT):
                s01 = smp.tile([128, 1], f32, tag="s01", name="s01")
                nc.vector.tensor_tensor(s01[:], tmps[0][ht][:], tmps[1][ht][:], add)
                for ci in range(2, len(tmps)):
                    nc.vector.tensor_tensor(s01[:], s01[:], tmps[ci][ht][:], add)
                nc.vector.tensor_scalar_mul(
                    ct_sb[:, ht * NLOC + n:ht * NLOC + n + 1], s01[:], prs[:, 0:1]
                )

        nc.sync.dma_start(c_out_d[:], ct_sb[:])

    return nc


_CACHE = {}


def _get_nc():
    if "nc" not in _CACHE:
        _CACHE["nc"] = _build()
    return _CACHE["nc"]


def kernel(out_e, hidden_d, W1, b1, W2, b2, W3, b3):
    out_e = np.ascontiguousarray(out_e, dtype=np.float32)
    hidden_d = np.ascontiguousarray(hidden_d, dtype=np.float32)
    W1 = np.asarray(W1, np.float32)
    W2 = np.asarray(W2, np.float32)
    W3 = np.asarray(W3, np.float32)
    b1 = np.asarray(b1, np.float32)
    b2 = np.asarray(b2, np.float32)

    # w2p[p, (jt*HT + ht)*128 + jj] = W2[jt*128+jj, ht*128+p]
    w2p = np.ascontiguousarray(
        W2.reshape(JT, 128, HT, 128).transpose(3, 0, 2, 1).reshape(128, JT * HT * 128)
    )
    w3p = np.ascontiguousarray(W3[0].reshape(JT, 128).T)               # [128, JT]
    # q = hd @ W1.T + b1 + b2 : [N, J] bias term (0.1% of total FLOPs)
    q_full = hidden_d[0] @ W1.T + (b1 + b2)[None, :]

    in_maps = []
    for i in range(NCORES):
        sl = slice(i * NLOC, (i + 1) * NLOC)
        ne = out_e[:, sl, :]                                           # [S, NLOC, H]
        # xT2[p, n, sc, ht, so] = ne[sc*512+so, n, ht*128+p]
        xT2 = np.ascontiguousarray(
            ne.reshape(2, 512, NLOC, HT, 128).transpose(4, 2, 0, 3, 1)
        ).reshape(128, NLOC * 2 * HT * 512)
        q = q_full[sl]                                                 # [NLOC, J]
        qt = np.ascontiguousarray(
            q.T.reshape(JT, 128, NLOC).transpose(1, 0, 2).reshape(128, JT * NLOC)
        )
        in_maps.append({"xT2": xT2, "w2p": w2p, "qt": qt, "w3p": w3p})

    nc = _get_nc()
    res = run_bass_kernel_spmd(nc, in_maps, core_ids=list(range(NCORES)))

    c = np.empty((1, N, H), np.float32)
    alpha = np.empty((S, N, 1), np.float32)
    for i in range(NCORES):
        ct = res.results[i]["c_out"]                                   # [128, HT*NLOC]
        c_i = ct.reshape(128, HT, NLOC).transpose(2, 1, 0).reshape(NLOC, H)
        c[0, i * NLOC:(i + 1) * NLOC, :] = c_i
        alpha[:, i * NLOC:(i + 1) * NLOC, 0] = res.results[i]["alpha_out"].T
    return c, alpha


# revision 59
# speedup vs baseline: 1.0276x; 1.0256x over previous
"""Trainium2 Bass kernel for Bahdanau-style attention.

Reference computation (per batch column n):
    e  = tanh(hd @ W1.T + b1 + out_e @ W2.T + b2)      # [S, N, J]
    a  = e @ W3.T + b3                                 # [S, N, 1]
    alpha = softmax(a, axis=0)                         # [S, N, 1]
    c  = einsum('sne,snh->enh', alpha, out_e)          # [1, N, H]
    returns (c, alpha)

Sharding: data-parallel over batch dim N (64) across 8 cores (8 cols each).

Device-side layout (per core): everything runs TRANSPOSED.  The host packs
out_e as xT2[p, (n, sc, ht, so)] so each (n, sc) unit is ONE 2MB DMA with
16KB-per-partition lines (a single dma_start fans out over all 16 DMA
engines), and the big matmul z.T = W2 @ x.T contracts h on the partition
dim with W2.T blocks stationary (packed jt-major, loaded in four DMAs so
the first j-tiles' matmuls start early).  All matmuls use float32r (full
fp32 storage, reduced-precision multiply, 1 cycle/row at free>=256 vs 4
for plain fp32; measured end-to-end error ~2e-4).  Throwaway matmuls
warm the PE HAM clock-gate during the startup DMA.
The q-term (hd @ W1.T + b1 + b2, a [N, J]-sized bias independent of the
big tensor, 0.1% of total FLOPs) is precomputed host-side and folded into
the tanh as a per-partition bias (partition dim = j in the transposed
layout; each 512-wide chunk has a single fixed n).  The a-projection
contracts j on partitions with W3 as a [128, 1] stationary.  softmax(a)
== softmax(a+b3) so b3 is dropped; the logits are O(1) so the
max-subtraction is skipped and the context einsum runs on unnormalized
exp weights per 512-chunk (overlapping the other chunk's matmuls),
normalized at the [128, 1] accumulator level at the end.  exp replication
across partitions uses a K=1 ones matmul; the einsum itself is a fused
DVE multiply+accumulate.
"""

import os
import sys

sys.path.insert(0, "/opt/trn_rl_repo")
os.environ.setdefault("MYCRO_LOCAL_CACHE", "1")

import ml_dtypes
import numpy as np

import bass_rust
import concourse.bass as bass
import concourse.mybir as mybir
import concourse.tile as tile
from concourse.bass_utils import run_bass_kernel_spmd

S, N, H, J = 1024, 64, 1024, 1024
NCORES = 8
NLOC = N // NCORES          # 8 batch columns per core
HT = H // 128               # 8 h-tiles
JT = J // 128               # 8 j-tiles
M = NLOC * S                # 8192 rows per core

f32 = mybir.dt.float32
f32r = mybir.dt.float32r
bf16 = mybir.dt.bfloat16

N_PROCS = bass_rust.N_PROCS


class ChunkedTileContext(tile.TileContext):
    """TileContext variant for a walrus build that rejects >1 sync wait per
    instruction: splits multi-wait instructions into single-wait NoOp
    carriers and chunks the kernel-tail drain into one drain per proc."""

    def _split_multiwaits(self, ordered):
        for bb in list(ordered.keys()):
            out = []
            for inst in ordered[bb]:
                si = inst.sync_info
                if si is not None and len(si.on_wait) > 1:
                    waits = list(si.on_wait)
                    for w in waits[:-1]:
                        nop = mybir.InstNoOp(
                            name=f"wsplit{self.nc.next_id()}",
                            engine=inst.engine,
                            sync_info=mybir.SyncInfo(on_wait=[w], on_update=[]),
                            bass_nofuse=True,
                        )
                        out.append(nop)
                    inst.sync_info = mybir.SyncInfo(
                        on_wait=[waits[-1]], on_update=list(si.on_update)
                    )
                out.append(inst)
            ordered[bb] = out

    def _lower_ordered_insts(self, ordered):
        self._split_multiwaits(ordered)
        return super()._lower_ordered_insts(ordered)

    def _drain_and_barrier(self, tick_clock, wait_clock):
        gc = tick_clock.global_clock
        vals = [gc.peek_next(p) - 1 for p in range(N_PROCS)]
        for p, v in enumerate(vals):
            if v <= 0:
                continue
            partial = [0] * N_PROCS
            partial[p] = v
            d = self.nc.sync.drain()
            wait_clock.add_sem_waits(
                d.ins,
                bass_rust.ScopedClock({None: bass_rust.VectorClock(partial)}),
            )
        self.nc.sync.drain()
        self.nc.all_engine_barrier()
        assert self.sems is not None
        popped = self.nc._tile_sem_poison_stack.pop()
        assert popped is self._sem_poison
        self.nc.clear_and_free_semaphores(list(self.sems.allocated().values()))
        self.nc.all_engine_barrier()


def _build():
    nc = bass.Bass()

    # xT2[p, ((n*2 + sc)*HT + ht)*512 + so] = out_e[sc*512+so, n, ht*128+p]
    xT_d = nc.declare_dram_parameter("xT2", [128, NLOC * 2 * HT * 512], f32r,
                                     isOutput=False)
    # w2p[p, (jt*HT + ht)*128 + jj] = W2[jt*128+jj, ht*128+p]
    w2p_d = nc.declare_dram_parameter("w2p", [128, JT * HT * 128], f32r,
                                      isOutput=False)
    qt_d = nc.declare_dram_parameter("qt", [128, JT * NLOC], f32, isOutput=False)
    w3p_d = nc.declare_dram_parameter("w3p", [128, JT], f32r, isOutput=False)
    c_out_d = nc.declare_dram_parameter("c_out", [128, HT * NLOC], f32, isOutput=True)
    alpha_d = nc.declare_dram_parameter("alpha_out", [NLOC, S], f32r, isOutput=True)
    tot_d = nc.declare_dram_parameter("tot_out", [NLOC, 8], f32, isOutput=True)

    Tanh = mybir.ActivationFunctionType.Tanh
    Exp = mybir.ActivationFunctionType.Exp
    mult = mybir.AluOpType.mult
    add = mybir.AluOpType.add

    with (
        ChunkedTileContext(nc) as tc,
        tc.tile_pool(name="const", bufs=1) as constp,
        tc.tile_pool(name="xtp", bufs=4) as xtp,
        tc.tile_pool(name="ttp", bufs=10) as ttp,
        tc.tile_pool(name="scrp", bufs=2) as scrp,
        tc.tile_pool(name="smp", bufs=2) as smp,
        tc.tile_pool(name="pz", bufs=5, space="PSUM") as pzp,
        tc.tile_pool(name="pa", bufs=1, space="PSUM") as pap,
        tc.tile_pool(name="pb", bufs=2, space="PSUM") as pbp,
    ):
        # ---------------- constants (tiny, arrive first) ----------------
        ones_f = constp.tile([1, 128], f32)
        nc.vector.memset(ones_f[:], 1.0)
        ones = constp.tile([1, 128], f32r)
        nc.vector.tensor_copy(ones[:], ones_f[:])
        qt_sb = constp.tile([128, JT * NLOC], f32)
        nc.sync.dma_start(qt_sb[:], qt_d[:])
        w3p_sb = constp.tile([128, JT], f32r)
        nc.sync.dma_start(w3p_sb[:], w3p_d[:])
        ct_sb = constp.tile([128, HT * NLOC], f32)

        # single big weight tile, jt-major; two DMA halves so jt 0-3 matmuls
        # can start while jt 4-7 weights stream
        w2_sb = constp.tile([128, JT * HT * 128], f32r)

        def w2blk(jt, ht):
            o = (jt * HT + ht) * 128
            return w2_sb[:, o:o + 128]

        # throwaway matmuls fill the PE during the startup DMA so the HAM
        # clock-gate is at 8/8 (and the pipe warm) when real data lands
        warm_in = constp.tile([128, 512], f32r)
        nc.vector.memset(warm_in[:].bitcast(f32), 0.0)
        pwarm = pzp.tile([128, 512], f32, tag="pz", name="pwarm")
        NWARM = 40
        for i in range(NWARM):
            nc.tensor.matmul(
                pwarm[:], warm_in[:, 0:128], warm_in[:],
                start=(i == 0), stop=(i == NWARM - 1),
            )

        # ---------------- main per-n pipeline ----------------
        for n in range(NLOC):
            xts = []
            for sc in range(2):
                x = xtp.tile([128, HT * 512], f32r, tag="xt", name="xt")
                nc.sync.dma_start(
                    x[:],
                    xT_d[:][:, (n * 2 + sc) * HT * 512:(n * 2 + sc + 1) * HT * 512],
                )
                xts.append(x)
                if n == 0 and sc == 0:
                    for quarter in range(4):
                        o = quarter * (JT // 4) * HT * 128
                        sz = (JT // 4) * HT * 128
                        nc.sync.dma_start(
                            w2_sb[:, o:o + sz], w2p_d[:][:, o:o + sz]
                        )

            # last n ends with two 256-wide chunks so the final post-matmul
            # (exp -> replicate -> einsum) tail is half as long.  Chunks in
            # the same group interleave per (jt, ht) so consecutive matmuls
            # share the stationary operand (saves ~12ns per shared reload);
            # n=0 keeps chunks in separate groups so the first group only
            # gates on the first 2MB of x.
            if n == NLOC - 1:
                groups = [[(0, 0, 512)], [(1, 0, 256)], [(1, 256, 256)]]
            else:
                groups = [[(0, 0, 512)], [(1, 0, 512)]]

            ses = []        # per-chunk (exp-values, exp-sum, base, width)
            tmps = []       # per-chunk list of per-ht unnormalized accums
            ci = 0
            for group in groups:
                all_tts = [[] for _ in group]
                for jt in range(JT):
                    pzs = [
                        pzp.tile([128, 512], f32, tag="pz", name="pz")
                        for _ in group
                    ]
                    for ht in range(HT):
                        for gi, (sc, off, w) in enumerate(group):
                            nc.tensor.matmul(
                                pzs[gi][:, 0:w],
                                w2blk(jt, ht),
                                xts[sc][:, ht * 512 + off:ht * 512 + off + w],
                                start=(ht == 0),
                                stop=(ht == HT - 1),
                            )
                    for gi, (sc, off, w) in enumerate(group):
                        tt = ttp.tile([128, 512], f32r, tag="tt", name="tt")
                        nc.scalar.activation(
                            tt[:, 0:w], pzs[gi][:, 0:w], Tanh,
                            bias=qt_sb[:, jt * NLOC + n:jt * NLOC + n + 1],
                            scale=1.0,
                        )
                        all_tts[gi].append(tt)
                for gi, (sc, off, w) in enumerate(group):
                    xsc = xts[sc]
                    tts = all_tts[gi]
                    pa = pap.tile([1, 512], f32, tag="pa", name="pa")
                    for jt in range(JT):
                        nc.tensor.matmul(
                            pa[:, 0:w],
                            w3p_sb[:, jt:jt + 1],
                            tts[jt][:, 0:w],
                            start=(jt == 0),
                            stop=(jt == JT - 1),
                        )
                    # unnormalized softmax weights: es = exp(a)  (logits are
                    # O(1): max-subtraction unnecessary; b3 cancels)
                    es = smp.tile([1, 512], f32r, tag="es", name="es", bufs=3)
                    se = smp.tile([1, 1], f32, tag="se", name="se", bufs=3)
                    nc.scalar.activation(
                        es[:, 0:w], pa[:, 0:w], Exp, scale=1.0, accum_out=se[:]
                    )
                    ses.append((es, se, sc * 512 + off, w))
                    # replicate es across 128 partitions via K=1 ones matmul
                    pb = pbp.tile([128, 512], f32, tag="pb", name="pb")
                    nc.tensor.matmul(
                        pb[:, 0:w], ones[:], es[:, 0:w], start=True, stop=True
                    )
                    # unnormalized context accumulation for this chunk
                    sc_tmps = []
                    for ht in range(HT):
                        tmp = smp.tile(
                            [128, 1], f32, tag=f"tmp{ci}", name="tmp", bufs=10
                        )
                        scr = scrp.tile([128, 512], f32, tag="scr", name="scr")
                        nc.vector.scalar_tensor_tensor(
                            out=scr[:, 0:w],
                            in0=xsc[:, ht * 512 + off:ht * 512 + off + w].bitcast(f32),
                            scalar=1.0, in1=pb[:, 0:w], op0=mult, op1=mult,
                            accum_out=tmp[:],
                        )
                        sc_tmps.append(tmp)
                    tmps.append(sc_tmps)
                    ci += 1

            # normalization happens host-side during unshard: output the
            # unnormalized exp rows, their sum, and unnormalized ct columns
            tot8 = smp.tile([1, 8], f32, tag="tot8", name="tot8")
            nc.vector.tensor_scalar(
                tot8[:], ones_f[:, 0:8], ses[0][1][:], ses[1][1][:],
                op0=mult, op1=add,
            )
            for es, se, _, _ in ses[2:]:
                nc.vector.tensor_scalar_add(tot8[:], tot8[:], se[:])
            nc.sync.dma_start(tot_d[:][n:n + 1, :], tot8[:])
            for es, se, base, w in ses:
                nc.sync.dma_start(
                    alpha_d[:][n:n + 1, base:base + w], es[:, 0:w]
                )
            for ht in range(HT):
                col = ct_sb[:, ht * NLOC + n:ht * NLOC + n + 1]
                if len(tmps) == 2:
                    nc.vector.tensor_tensor(col, tmps[0][ht][:], tmps[1][ht][:], add)
                else:
                    s01 = smp.tile([128, 1], f32, tag="s01", name="s01")
                    nc.vector.tensor_tensor(
                        s01[:], tmps[0][ht][:], tmps[1][ht][:], add
                    )
                    nc.vector.tensor_tensor(col, s01[:], tmps[2][ht][:], add)

        nc.sync.dma_start(c_out_d[:], ct_sb[:])

    return nc


_CACHE = {}


def _get_nc():
    if "nc" not in _CACHE:
        _CACHE["nc"] = _build()
    return _CACHE["nc"]


def kernel(out_e, hidden_d, W1, b1, W2, b2, W3, b3):
    out_e = np.ascontiguousarray(out_e, dtype=np.float32)
    hidden_d = np.ascontiguousarray(hidden_d, dtype=np.float32)
    W1 = np.asarray(W1, np.float32)
    W2 = np.asarray(W2, np.float32)
    W3 = np.asarray(W3, np.float32)
    b1 = np.asarray(b1, np.float32)
    b2 = np.asarray(b2, np.float32)

    # w2p[p, (jt*HT + ht)*128 + jj] = W2[jt*128+jj, ht*128+p]
    w2p = np.ascontiguousarray(
        W2.reshape(JT, 128, HT, 128).transpose(3, 0, 2, 1).reshape(128, JT * HT * 128)
    )
    w3p = np.ascontiguousarray(W3[0].reshape(JT, 128).T)               # [128, JT]
    # q = hd @ W1.T + b1 + b2 : [N, J] bias term (0.1% of total FLOPs)
    q_full = hidden_d[0] @ W1.T + (b1 + b2)[None, :]

    in_maps = []
    for i in range(NCORES):
        sl = slice(i * NLOC, (i + 1) * NLOC)
        ne = out_e[:, sl, :]                                           # [S, NLOC, H]
        # xT2[p, n, sc, ht, so] = ne[sc*512+so, n, ht*128+p]
        xT2 = np.ascontiguousarray(
            ne.reshape(2, 512, NLOC, HT, 128).transpose(4, 2, 0, 3, 1)
        ).reshape(128, NLOC * 2 * HT * 512)
        q = q_full[sl]                                                 # [NLOC, J]
        qt = np.ascontiguousarray(
            q.T.reshape(JT, 128, NLOC).transpose(1, 0, 2).reshape(128, JT * NLOC)
        )
        in_maps.append({"xT2": xT2, "w2p": w2p, "qt": qt, "w3p": w3p})

    nc = _get_nc()
    res = run_bass_kernel_spmd(nc, in_maps, core_ids=list(range(NCORES)))

    c = np.empty((1, N, H), np.float32)
    alpha = np.empty((S, N, 1), np.float32)
    for i in range(NCORES):
        inv = 1.0 / res.results[i]["tot_out"][:, 0]                    # [NLOC]
        ct = res.results[i]["c_out"]                                   # [128, HT*NLOC]
        c_i = ct.reshape(128, HT, NLOC).transpose(2, 1, 0).reshape(NLOC, H)
        c[0, i * NLOC:(i + 1) * NLOC, :] = c_i * inv[:, None]
        alpha[:, i * NLOC:(i + 1) * NLOC, 0] = (
            res.results[i]["alpha_out"] * inv[:, None]
        ).T
    return c, alpha
